# revision 2
# baseline (speedup 1.0000x reference)
"""Single-head attention (B=8, S=2048, H=1024, D=64) on 8 TRN2 NeuronCores.

Sharding: data-parallel over the batch dim - core b computes batch element b.

v2: bf16 datapath. All PE operands are bf16 (f32 PSUM accumulation), which
halves SBUF traffic, enables packed 2x DVE evacuations, and cuts transpose
cost on the PE (1.0 cyc/col vs 1.5 for f32r). Error budget: bf16 quantization
contributes ~0.5-1% rel err vs the 2e-2 gate.

Per-core dataflow:
  x tiles [128s, 1024h] f32  --DMA-->  xs  --DVE-->  x16 (bf16)
  xT[h, s]     = PE-transpose of x16 tiles (bf16 -> bf16 PSUM, packed evac)
  qT/kT        = wqk.T @ xT (+bias on evac)          (bf16, [128=(q|k), S])
  stack_kq     = swapped copy of stack_qk (DMA) so both PE row-halves
                 see both tensors for the 64x128 row-split scores
  vT           = wv.T @ xT (+bias)                   (bf16, [64, S])
  v_aug        = [v | mask | 0pad] in [k, 96] layout (bf16; masked keys
                 contribute 0 to numerator AND denominator)
  scoresT      = row-split PE (T0/T8 pairs into separate PSUM banks)
  attnT        = exp(scoresT/8) on ScalarE -> bf16   (the pacing engine)
  outT_aug     = sum_k v_aug.T @ attnT (f32 PSUM accum, [96, 512])
  out          = transpose(outT_aug), rows /= row 64 (denominator), one
                 [2048, 64] f32 store per iteration
"""

import sys

sys.path.insert(0, "/opt/trn_rl_repo")

import numpy as np

B, S, H, D = 8, 2048, 1024, 64
SB = 512          # s-block (streaming block of queries)
NBLK = S // SB    # 4
NT = S // 128     # 16 t-tiles (and s-tiles)
HC = H // 128     # 8 h-chunks


def build_nc(repeats=1):
    import concourse.bacc as bacc
    import concourse.mybir as mybir
    import concourse.tile as tile
    from concourse.masks import make_identity

    dt = mybir.dt
    f32, bf16, i32 = dt.float32, dt.bfloat16, dt.int32
    AF = mybir.ActivationFunctionType

    nc = bacc.Bacc("TRN2", target_bir_lowering=False, debug=False, num_devices=8)

    X = nc.dram_tensor("x_b", [S, H], f32, kind="ExternalInput")
    MASK = nc.dram_tensor("mask_b", [S], i32, kind="ExternalInput")
    WQ = nc.dram_tensor("Wq", [H, D], f32, kind="ExternalInput")
    BQ = nc.dram_tensor("bq", [D], f32, kind="ExternalInput")
    WK = nc.dram_tensor("Wk", [H, D], f32, kind="ExternalInput")
    BK = nc.dram_tensor("bk", [D], f32, kind="ExternalInput")
    WV = nc.dram_tensor("Wv", [H, D], f32, kind="ExternalInput")
    BV = nc.dram_tensor("bv", [D], f32, kind="ExternalInput")
    OUT = nc.dram_tensor("out_b", [S, D], f32, kind="ExternalOutput")

    with tile.TileContext(nc) as tc:
        with (
            tc.tile_pool(name="const", bufs=1) as cpool,
            tc.tile_pool(name="xs", bufs=2) as xs_pool,
            tc.tile_pool(name="x16", bufs=2) as x16_pool,
            tc.tile_pool(name="xt", bufs=2) as xt_pool,
            tc.tile_pool(name="stk", bufs=2) as stk_pool,
            tc.tile_pool(name="vps", bufs=2) as v_pool,
            tc.tile_pool(name="attn", bufs=2) as at_pool,
            tc.tile_pool(name="outs", bufs=2) as o_pool,
            tc.tile_pool(name="ps_tr", bufs=2, space="PSUM") as ps_tr,
            tc.tile_pool(name="ps_sc", bufs=2, space="PSUM") as ps_sc,
            tc.tile_pool(name="ps_acc", bufs=2, space="PSUM") as ps_acc,
        ):
            # ---- constants ----
            ident_f = cpool.tile([128, 128], f32)
            make_identity(nc, ident_f)
            ident = cpool.tile([128, 128], bf16)
            nc.vector.tensor_copy(ident, ident_f)

            # fused [Wq | Wk] stationary (bf16): one projection matmul makes q and k
            wstage = cpool.tile([128, HC, 2 * D], f32, tag="wstage")
            nc.gpsimd.dma_start(out=wstage[:, :, 0:D], in_=WQ.ap().rearrange("(c p) m -> p c m", p=128))
            nc.gpsimd.dma_start(out=wstage[:, :, D:2 * D], in_=WK.ap().rearrange("(c p) m -> p c m", p=128))
            wqk = cpool.tile([128, HC, 2 * D], bf16)
            nc.vector.tensor_copy(wqk, wstage)
            wstage2 = cpool.tile([128, HC, 2 * D], f32, tag="wstage")
            nc.gpsimd.dma_start(out=wstage2[:, :, 0:D], in_=WV.ap().rearrange("(c p) m -> p c m", p=128))
            wv = cpool.tile([128, HC, D], bf16)
            nc.vector.tensor_copy(wv, wstage2[:, :, 0:D])

            bias_qk = cpool.tile([128, 1], f32)
            bias_v = cpool.tile([D, 1], f32)
            nc.gpsimd.dma_start(out=bias_qk[0:D, :], in_=BQ.ap().rearrange("(p o) -> p o", o=1))
            nc.gpsimd.dma_start(out=bias_qk[D:2 * D, :], in_=BK.ap().rearrange("(p o) -> p o", o=1))
            nc.gpsimd.dma_start(out=bias_v, in_=BV.ap().rearrange("(p o) -> p o", o=1))

            # key mask as 1.0/0.0 per t-chunk column (folded into v_aug)
            mask_i = cpool.tile([128, NT], i32)
            nc.gpsimd.dma_start(out=mask_i, in_=MASK.ap().rearrange("(c p) -> p c", p=128))
            mask_f = cpool.tile([128, NT], f32)
            nc.vector.tensor_copy(mask_f, mask_i)
            mask_m = cpool.tile([128, NT], f32)
            nc.vector.tensor_scalar(
                out=mask_m, in0=mask_f,
                scalar1=0.0, scalar2=None,
                op0=mybir.AluOpType.not_equal,
            )

            # v_aug [k-part, t-tile, 96]: cols 0:64 = masked v (rewritten each
            # iteration), col 64 = mask (the softmax-denominator ones column),
            # cols 65:96 = zero pad. Cols 64:96 are written ONCE here: bufs=1
            # keeps the slot stable and per-iteration writes only touch 0:64.
            v_aug = cpool.tile([128, NT, 96], bf16, tag="v_aug")
            nc.vector.memset(v_aug[:, :, D + 1:96], 0.0)
            for i in range(NT):
                nc.vector.tensor_copy(v_aug[:, i, D:D + 1], mask_m[:, i:i + 1])

            for _rep in range(repeats):
                # ---- stage A: load x, convert to bf16, transpose to xT ----
                # xt layout [128, c, st, 128]: st-major matches the packed
                # transpose-group PSUM tiles for single-instruction evacs
                xt = xt_pool.tile([128, HC, NT, 128], bf16, tag="xt")
                for jb in range(NBLK):
                    xs = xs_pool.tile([128, 4, H], f32, tag="xs")
                    nc.sync.dma_start(
                        out=xs,
                        in_=X.ap().rearrange("(b t p) h -> p b t h", p=128, b=NBLK)[:, jb, :, :],
                    )
                    x16 = x16_pool.tile([128, 4, H], bf16, tag="x16")
                    nc.vector.tensor_copy(x16, xs)
                    for c in range(HC):
                        ps = ps_tr.tile([128, 4, 128], bf16, tag="tr")
                        for st in range(4):
                            nc.tensor.transpose(
                                ps[:, st, :], x16[:, st, c * 128:(c + 1) * 128], ident
                            )
                        nc.vector.tensor_copy(xt[:, c, jb * 4:(jb + 1) * 4, :], ps)

                # ---- stage B: qk projection (+bias), build swapped stack ----
                stack_qk = stk_pool.tile([128, S], bf16, tag="stack_qk")
                stack_kq = stk_pool.tile([128, S], bf16, tag="stack_kq")
                vT = v_pool.tile([D, S], bf16, tag="vT")
                for j in range(NBLK):
                    sl = slice(j * SB, (j + 1) * SB)
                    ps = ps_acc.tile([128, SB], f32, tag="acc")
                    for c in range(HC):
                        nc.tensor.matmul(
                            ps, wqk[:, c, :], xt[:, c, j * 4:(j + 1) * 4, :],
                            start=(c == 0), stop=(c == HC - 1),
                        )
                    nc.vector.tensor_scalar_add(stack_qk[:, sl], ps, bias_qk)
                    nc.sync.dma_start(out=stack_kq[0:D, sl], in_=stack_qk[D:2 * D, sl])
                    nc.sync.dma_start(out=stack_kq[D:2 * D, sl], in_=stack_qk[0:D, sl])

                # ---- stage C: attention per s-block ----
                def pass1(jb):
                    # scores on the row-split PE + exp for block jb
                    sl = slice(jb * SB, (jb + 1) * SB)
                    at = at_pool.tile([128, NT // 2, 2, SB], bf16, tag="at")
                    for ih in range(NT // 2):
                        ps = ps_sc.tile([128, 2, SB], f32, tag="sc")
                        i0, i1 = ih, ih + NT // 2
                        nc.tensor.matmul(
                            ps[:, 0, :],
                            stack_kq[0:D, i0 * 128:(i0 + 1) * 128],
                            stack_qk[0:D, sl],
                            start=True, stop=True, tile_position=(0, 0),
                        )
                        nc.tensor.matmul(
                            ps[:, 1, :],
                            stack_qk[D:2 * D, i1 * 128:(i1 + 1) * 128],
                            stack_kq[D:2 * D, sl],
                            start=True, stop=True, tile_position=(64, 0),
                        )
                        nc.scalar.activation(
                            out=at[:, ih, :, :], in_=ps, func=AF.Exp, scale=0.125,
                        )
                    return at

                def project_v():
                    for j in range(NBLK):
                        sl = slice(j * SB, (j + 1) * SB)
                        ps_v = ps_acc.tile([128, SB], f32, tag="acc")
                        for c in range(HC):
                            nc.tensor.matmul(
                                ps_v[0:D, :], wv[:, c, :], xt[:, c, j * 4:(j + 1) * 4, :],
                                start=(c == 0), stop=(c == HC - 1),
                            )
                        nc.vector.tensor_scalar_add(vT[:, sl], ps_v[0:D, :], bias_v)
                        pst = ps_tr.tile([128, 4, 128], bf16, tag="tr")
                        for st in range(4):
                            i = j * 4 + st
                            nc.tensor.transpose(
                                pst[:, st, 0:D], vT[:, i * 128:(i + 1) * 128], ident[0:D, 0:D]
                            )
                        for st in range(4):
                            i = j * 4 + st
                            nc.vector.tensor_scalar_mul(
                                v_aug[:, i, 0:D], pst[:, st, 0:D], mask_m[:, i:i + 1]
                            )

                def pass2(jb, at):
                    # attn@v + transpose + normalize for block jb
                    ps_o = ps_acc.tile([128, SB], f32, tag="acc")
                    for i in range(NT):
                        nc.tensor.matmul(
                            ps_o[0:96, :], v_aug[:, i, :],
                            at[:, i % (NT // 2), i // (NT // 2), :],
                            start=(i == 0), stop=(i == NT - 1),
                        )
                    o_t = o_pool.tile([96, SB], bf16, tag="ot")
                    nc.vector.tensor_copy(o_t, ps_o[0:96, :])
                    pst = ps_tr.tile([128, 4, 96], bf16, tag="tr")
                    for st in range(4):
                        nc.tensor.transpose(
                            pst[:, st, :], o_t[:, st * 128:(st + 1) * 128], ident[0:96, 0:96]
                        )
                    for st in range(4):
                        recip = o_pool.tile([128, 1], f32, tag="recip")
                        nc.vector.reciprocal(recip, pst[:, st, D:D + 1])
                        nc.vector.tensor_scalar_mul(
                            outbuf[:, jb * 4 + st, :], pst[:, st, 0:D], recip
                        )

                outbuf = o_pool.tile([128, NT, D], f32, tag="outbuf")
                # software pipeline; v projection emitted after block 0's
                # scores so the PE does v work under the first exp wavefront
                prev = None
                for jb in range(NBLK):
                    at = pass1(jb)
                    if jb == 0:
                        project_v()
                    if prev is not None:
                        pass2(prev[0], prev[1])
                    prev = (jb, at)
                pass2(prev[0], prev[1])
                nc.sync.dma_start(
                    out=OUT.ap().rearrange("(t p) d -> p t d", p=128), in_=outbuf
                )

    nc.compile()
    return nc


_NC = None


def kernel(x, mask, Wq, bq, Wk, bk, Wv, bv):
    global _NC
    if _NC is None:
        _NC = build_nc()
    from concourse.bass_utils import run_bass_kernel_spmd

    x = np.ascontiguousarray(np.asarray(x, dtype=np.float32))
    mask = np.ascontiguousarray(np.asarray(mask, dtype=np.int32))
    shared = {
        "Wq": np.asarray(Wq, np.float32), "bq": np.asarray(bq, np.float32),
        "Wk": np.asarray(Wk, np.float32), "bk": np.asarray(bk, np.float32),
        "Wv": np.asarray(Wv, np.float32), "bv": np.asarray(bv, np.float32),
    }
    in_maps = [dict(x_b=x[c], mask_b=mask[c], **shared) for c in range(B)]
    # the device occasionally wedges transiently (NRT_EXEC_UNIT_UNRECOVERABLE);
    # a retry on a fresh execution recovers it
    last_err = None
    for attempt in range(3):
        try:
            res = run_bass_kernel_spmd(_NC, in_maps, core_ids=list(range(B)))
            return np.stack([res.results[c]["out_b"] for c in range(B)], axis=0)
        except Exception as e:  # noqa: BLE001
            last_err = e
            import time as _time

            _time.sleep(2.0 * (attempt + 1))
    raise last_err
